# revision 1
# baseline (speedup 1.0000x reference)
# Trainium2 Bass kernel: MultiHeadCrossAttentionLayer
#
# Sharding: data-parallel over batch. B=8 -> one batch element per NeuronCore,
# no collectives; gather = np.stack on host.
#
# Per-core computation (batch element b):
#   z   = standardize(query)            (LN; gamma/beta folded into Wq on host)
#   qT  = Wq_eff @ z.T + bq_eff         [768, 512]  (o on partitions)
#   kT  = Wk.T.T... = Wk @ key.T        [768, 4096] via PE-transposed key chunks
#   v   = value @ Wv.T                  [4096, 768] (kv on partitions)
#   S_T[kv, q] = kT_h.T-slices @ qT_h   per head; exp on ACT (no max needed:
#                scores ~ N(0,1), |S|max ~ 6), multiplicative mask on DVE
#   O_T[o', t] accumulated in PSUM over kv; row-sums r via ones-matmuls
#   out = (O_T / r).T @ Wo.T + cvec     (bias+norm folded; cvec = bv@Wo.T+bo)
#
# k-bias is dropped (softmax-invariant: adds a per-q constant to scores).

import functools
import numpy as np

B = 8
Q = 512
KV = 4096
D = 768
H = 12
DH = 64

NCHUNK = 16          # kv chunks for K/V projection (tokens per chunk = 256)
CTOK = KV // NCHUNK  # 256
NSUB = KV // 128     # 32 kv sub-chunks of 128 for attention
NG = 3               # head groups
GH = 4               # heads per group


def _f32(x):
    return np.ascontiguousarray(np.asarray(x, dtype=np.float32))


def _bf16(x):
    import ml_dtypes
    return np.ascontiguousarray(np.asarray(x, dtype=np.float32).astype(ml_dtypes.bfloat16))


@functools.lru_cache(maxsize=1)
def _build():
    import concourse.bass as bass
    import concourse.tile as tile
    from concourse import bacc, mybir
    from concourse.masks import make_identity

    fp32 = mybir.dt.float32
    bf16 = mybir.dt.bfloat16
    i32 = mybir.dt.int32
    AF = mybir.ActivationFunctionType
    ALU = mybir.AluOpType

    nc = bacc.Bacc(None, target_bir_lowering=False)

    names = {}

    with tile.TileContext(nc) as tc:
        with tc.tile_pool(name="dram", bufs=1, space="DRAM") as dram:
            d_query = dram.tile([Q, D], fp32, kind="ExternalInput")
            d_key = dram.tile([KV, D], fp32, kind="ExternalInput")
            d_value = dram.tile([KV, D], fp32, kind="ExternalInput")
            d_mask = dram.tile([Q, KV], i32, kind="ExternalInput")
            d_wqT = dram.tile([D, D], bf16, kind="ExternalInput")
            d_wkT = dram.tile([D, D], bf16, kind="ExternalInput")
            d_wvT = dram.tile([D, D], bf16, kind="ExternalInput")
            d_woT = dram.tile([D, D], bf16, kind="ExternalInput")
            d_bq = dram.tile([128, 6], fp32, kind="ExternalInput")
            d_cvec = dram.tile([1, D], fp32, kind="ExternalInput")
            d_out = dram.tile([Q, D], fp32, kind="ExternalOutput")
            d_rscratch = dram.tile([H, Q], fp32)

            names = dict(
                query=d_query.name, key=d_key.name, value=d_value.name,
                mask=d_mask.name, wqT=d_wqT.name, wkT=d_wkT.name,
                wvT=d_wvT.name, woT=d_woT.name, bq=d_bq.name,
                cvec=d_cvec.name, out=d_out.name,
            )

            import os
            debug = os.environ.get("BASSDBG", "0") == "1"
            if debug:
                d_dbg_qT = dram.tile([D, Q], bf16, kind="ExternalOutput")
                d_dbg_kT = dram.tile([128, KV], bf16, kind="ExternalOutput")
                d_dbg_v = dram.tile([128, D], bf16, kind="ExternalOutput")
                d_dbg_keep = dram.tile([128, Q], bf16, kind="ExternalOutput")
                d_dbg_p4 = dram.tile([128, 4 * Q], bf16, kind="ExternalOutput")
                d_dbg_oT = dram.tile([D, Q], bf16, kind="ExternalOutput")
                d_dbg_r = dram.tile([128, Q], fp32, kind="ExternalOutput")
                names.update(dbg_qT=d_dbg_qT.name, dbg_kT=d_dbg_kT.name,
                             dbg_v=d_dbg_v.name, dbg_keep=d_dbg_keep.name,
                             dbg_p4=d_dbg_p4.name, dbg_oT=d_dbg_oT.name,
                             dbg_r=d_dbg_r.name)

            # ---------------- persistent SBUF ----------------
            persist_cm = tc.tile_pool(name="persist", bufs=1)
            persist = persist_cm.__enter__()
            ident = persist.tile([128, 128], bf16)
            make_identity(nc, ident)
            ident_f = persist.tile([128, 128], fp32)
            make_identity(nc, ident_f)
            ones_col = persist.tile([128, 1], bf16)
            nc.vector.memset(ones_col, 1.0)
            bq_sb = persist.tile([128, 6], fp32)
            nc.sync.dma_start(out=bq_sb, in_=d_bq[:])
            cvec_sb = persist.tile([128, D], fp32)
            cv_ap = d_cvec[:]
            nc.gpsimd.dma_start(
                out=cvec_sb,
                in_=bass.AP(tensor=cv_ap.tensor, offset=cv_ap.offset,
                            ap=[[0, 128]] + list(cv_ap.ap[1:])),
            )

            # kT_sb: [768, 4096] bf16 as 6 tiles; v_sb: [4096, 768] as 32 tiles
            kT_sb = [persist.tile([128, KV], bf16, tag=f"kT{i}", name=f"kT{i}") for i in range(6)]
            v_sb = [persist.tile([128, D], bf16, tag=f"v{i}", name=f"v{i}") for i in range(NSUB)]
            qT_sb = [persist.tile([128, Q], bf16, tag=f"qT{i}", name=f"qT{i}") for i in range(6)]

            # =============== Phase A: LN(query) -> z.T -> qT ===============
            scopeA = nc.named_scope("phaseA_ln_q"); scopeA.__enter__()
            with (
                tc.tile_pool(name="phA", bufs=2) as phA,
                tc.tile_pool(name="phA1", bufs=1) as phA1,
                tc.tile_pool(name="phA_ps", bufs=2, space="PSUM") as phA_ps,
                tc.tile_pool(name="phA_ps2", bufs=2, space="PSUM") as phA_ps2,
            ):
                wqT_sb = [phA1.tile([128, D], bf16, tag=f"wq{i}", name=f"wq{i}") for i in range(6)]
                for i in range(6):
                    nc.sync.dma_start(out=wqT_sb[i], in_=d_wqT[128 * i:128 * (i + 1), :])
                eps_sb = phA1.tile([128, 1], fp32)
                nc.vector.memset(eps_sb, 1e-5)

                z_bf = [phA1.tile([128, D], bf16, tag=f"z{c}", name=f"z{c}") for c in range(4)]
                q_nat = d_query[:].rearrange("(c p) d -> p c d", p=128)
                for c in range(4):
                    x_t = phA.tile([128, D], fp32, tag="lnx")
                    nc.sync.dma_start(out=x_t, in_=q_nat[:, c, :])
                    # bn_stats needs free dim <= 512; use 3 subgroups of 256
                    stats = phA.tile([128, 3, 6], fp32, tag="lnst")
                    xg = x_t[:].rearrange("p (s d) -> p s d", s=3)
                    for s in range(3):
                        nc.vector.bn_stats(out=stats[:, s, :], in_=xg[:, s, :])
                    mv = phA.tile([128, 2], fp32, tag="lnmv")
                    nc.vector.bn_aggr(out=mv, in_=stats[:])
                    rstd = phA.tile([128, 1], fp32, tag="lnrs")
                    nc.scalar.activation(out=rstd, in_=mv[:, 1:2], func=AF.Sqrt,
                                         bias=eps_sb[:], scale=1.0)
                    nc.vector.reciprocal(out=rstd, in_=rstd)
                    nc.vector.tensor_scalar(
                        out=z_bf[c][:], in0=x_t[:], scalar1=mv[:, 0:1],
                        scalar2=rstd[:], op0=ALU.subtract, op1=ALU.mult,
                    )
                # transpose z -> zT (6 tiles [128, 512])
                zT = [phA1.tile([128, Q], bf16, tag=f"zT{i}", name=f"zT{i}") for i in range(6)]
                for i in range(6):
                    ps = phA_ps.tile([128, Q], bf16, tag="zTps")
                    for c in range(4):
                        nc.tensor.transpose(
                            out=ps[:, 128 * c:128 * (c + 1)],
                            in_=z_bf[c][:, 128 * i:128 * (i + 1)],
                            identity=ident[:],
                        )
                    nc.vector.tensor_copy(out=zT[i][:], in_=ps[:])
                # qT[o,:] = sum_i WqT[i, o-block].T @ zT[i]
                for o in range(6):
                    ps = phA_ps2.tile([128, Q], fp32, tag="qps")
                    for i in range(6):
                        nc.tensor.matmul(
                            out=ps[:], lhsT=wqT_sb[i][:, 128 * o:128 * (o + 1)],
                            rhs=zT[i][:], start=(i == 0), stop=(i == 5),
                        )
                    nc.vector.tensor_scalar(
                        out=qT_sb[o][:], in0=ps[:], scalar1=bq_sb[:, o:o + 1],
                        scalar2=None, op0=ALU.add,
                    )

            if debug:
                for i in range(6):
                    nc.sync.dma_start(out=d_dbg_qT[128 * i:128 * (i + 1), :],
                                      in_=qT_sb[i][:])

            scopeA.__exit__(None, None, None)
            # =============== Phase B: K/V projections ===============
            scopeB = nc.named_scope("phaseB_kvproj"); scopeB.__enter__()
            with (
                tc.tile_pool(name="phB1", bufs=1) as phB1,
                tc.tile_pool(name="stage", bufs=2) as stage,
                tc.tile_pool(name="tchunk", bufs=2) as tchunk,
                tc.tile_pool(name="phB_tp", bufs=3, space="PSUM") as phB_tp,
                tc.tile_pool(name="phB_k", bufs=2, space="PSUM") as phB_k,
                tc.tile_pool(name="phB_v", bufs=2, space="PSUM") as phB_v,
            ):
                wkT_sb = [phB1.tile([128, D], bf16, tag=f"wk{i}", name=f"wk{i}") for i in range(6)]
                wvT_sb = [phB1.tile([128, D], bf16, tag=f"wv{i}", name=f"wv{i}") for i in range(6)]
                for i in range(6):
                    nc.sync.dma_start(out=wkT_sb[i], in_=d_wkT[128 * i:128 * (i + 1), :])
                    nc.sync.dma_start(out=wvT_sb[i], in_=d_wvT[128 * i:128 * (i + 1), :])

                key_nat = d_key[:].rearrange("(c s p) d -> c p s d", p=128, s=CTOK // 128)
                val_nat = d_value[:].rearrange("(c s p) d -> c p s d", p=128, s=CTOK // 128)
                NS = CTOK // 128  # 2 sub-tiles of 128 tokens per chunk

                for c in range(NCHUNK):
                    kf = stage.tile([128, NS, D], fp32, tag="kf32")
                    vf = stage.tile([128, NS, D], fp32, tag="vf32")
                    nc.sync.dma_start(out=kf, in_=key_nat[c])
                    nc.sync.dma_start(out=vf, in_=val_nat[c])
                    # transpose chunks (f32 in, bf16 out via evict): [128(i), 6, CTOK]
                    keyT_c = tchunk.tile([128, 6, CTOK], bf16, tag="keyT")
                    valT_c = tchunk.tile([128, 6, CTOK], bf16, tag="valT")
                    for src, dstT in ((kf, keyT_c), (vf, valT_c)):
                        for i2 in range(0, 6, 2):  # two i-blocks -> one psum tile
                            ps = phB_tp.tile([128, 2, NS, 128], fp32, tag="tps")
                            for di in range(2):
                                for s in range(NS):
                                    nc.tensor.transpose(
                                        out=ps[:, di, s, :],
                                        in_=src[:, s, 128 * (i2 + di):128 * (i2 + di + 1)],
                                        identity=ident_f[:],
                                    )
                            nc.vector.tensor_copy(
                                out=dstT[:, i2:i2 + 2, :]
                                    .rearrange("p a (s k) -> p a s k", s=NS),
                                in_=ps[:],
                            )

                    # kT += WkT.T-blocks @ keyT_c
                    for o in range(6):
                        ps = phB_k.tile([128, CTOK], fp32, tag="kps")
                        for i in range(6):
                            nc.tensor.matmul(
                                out=ps[:], lhsT=wkT_sb[i][:, 128 * o:128 * (o + 1)],
                                rhs=keyT_c[:, i, :], start=(i == 0), stop=(i == 5),
                            )
                        nc.vector.tensor_copy(
                            out=kT_sb[o][:, CTOK * c:CTOK * (c + 1)], in_=ps[:])
                    # v rows: v[t, :] = valT_c.T-blocks @ WvT
                    for s in range(NS):
                        for half in range(2):
                            ps = phB_v.tile([128, 384], fp32, tag="vps")
                            for i in range(6):
                                nc.tensor.matmul(
                                    out=ps[:],
                                    lhsT=valT_c[:, i, 128 * s:128 * (s + 1)],
                                    rhs=wvT_sb[i][:, 384 * half:384 * (half + 1)],
                                    start=(i == 0), stop=(i == 5),
                                )
                            nc.vector.tensor_copy(
                                out=v_sb[NS * c + s][:, 384 * half:384 * (half + 1)],
                                in_=ps[:])

            if debug:
                nc.sync.dma_start(out=d_dbg_kT[:], in_=kT_sb[0][:])
                nc.sync.dma_start(out=d_dbg_v[:], in_=v_sb[0][:])

            scopeB.__exit__(None, None, None)
            # =============== Phase C: keepT = (1 - mask).T ===============
            scopeC = nc.named_scope("phaseC_mask"); scopeC.__enter__()
            keepT = [persist.tile([128, Q], bf16, tag=f"keep{s}", name=f"keep{s}") for s in range(NSUB)]
            with (
                tc.tile_pool(name="phC", bufs=3) as phC,
                tc.tile_pool(name="phC_ps", bufs=1, space="PSUM") as phC_ps,
            ):
                for s in range(NSUB):
                    mraw = phC.tile([128, 4, 128], i32, tag="mraw")
                    nc.sync.dma_start(
                        out=mraw,
                        in_=d_mask[:, 128 * s:128 * (s + 1)]
                            .rearrange("(c p) k -> p c k", p=128),
                    )
                    mb = phC.tile([128, 4, 128], bf16, tag="mbf")
                    nc.vector.tensor_scalar(
                        out=mb[:], in0=mraw[:], scalar1=-1.0, scalar2=1.0,
                        op0=ALU.mult, op1=ALU.add,
                    )
                    ps = phC_ps.tile([128, 4, 128], bf16, tag="cps")
                    for c in range(4):
                        nc.tensor.transpose(out=ps[:, c, :], in_=mb[:, c, :],
                                            identity=ident[:])
                    nc.vector.tensor_copy(
                        out=keepT[s][:].rearrange("p (c k) -> p c k", c=4), in_=ps[:])

            if debug:
                nc.sync.dma_start(out=d_dbg_keep[:], in_=keepT[0][:])

            scopeC.__exit__(None, None, None)
            # =============== Phase D: attention (+ E interleaved) ===============
            scopeD = nc.named_scope("phaseD_attn"); scopeD.__enter__()
            with (
                tc.tile_pool(name="phD", bufs=3) as phD,
                tc.tile_pool(name="phD_s", bufs=1, space="PSUM") as phD_s,
                tc.tile_pool(name="phD_o", bufs=1, space="PSUM") as phD_o,
                tc.tile_pool(name="phD_r", bufs=1, space="PSUM") as phD_r,
                tc.tile_pool(name="phE", bufs=2) as phE,
                tc.tile_pool(name="phE1", bufs=1) as phE1,
                tc.tile_pool(name="phE_ps", bufs=1, space="PSUM") as phE_ps,
            ):
                woT_sb = [phE1.tile([128, D], bf16, tag=f"wo{i}", name=f"wo{i}") for i in range(6)]
                for i in range(6):
                    nc.sync.dma_start(out=woT_sb[i], in_=d_woT[128 * i:128 * (i + 1), :])
                oT_sb = [persist.tile([128, Q], bf16, tag=f"oT{i}", name=f"oT{i}") for i in range(6)]
                for g in range(NG):
                    o01 = phD_o.tile([128, Q], fp32, tag="o01")
                    o23 = phD_o.tile([128, Q], fp32, tag="o23")
                    opair = (o01, o23)
                    r_ps = phD_r.tile([128, Q], fp32, tag="rps")
                    s4 = phD_s.tile([128, 4, Q], fp32, tag="s4")
                    kt0 = kT_sb[2 * g]      # heads 4g, 4g+1
                    kt1 = kT_sb[2 * g + 1]  # heads 4g+2, 4g+3
                    qt0 = qT_sb[2 * g]
                    qt1 = qT_sb[2 * g + 1]
                    for s in range(NSUB):
                        sl = slice(128 * s, 128 * (s + 1))
                        for hh, (kt, qt) in enumerate(((kt0, qt0), (kt1, qt1))):
                            for j in range(2):
                                nc.tensor.matmul(
                                    out=s4[:, 2 * hh + j, :],
                                    lhsT=kt[64 * j:64 * (j + 1), sl],
                                    rhs=qt[64 * j:64 * (j + 1), :],
                                    start=True, stop=True,
                                    tile_position=(64 * j, 0),
                                )
                        e4 = phD.tile([128, 4, Q], bf16, tag="e4")
                        nc.scalar.activation(out=e4[:], in_=s4[:], func=AF.Exp)
                        p4 = phD.tile([128, 4, Q], bf16, tag="p4")
                        kap = keepT[s][:]
                        nc.vector.tensor_mul(
                            p4[:], e4[:],
                            bass.AP(tensor=kap.tensor, offset=kap.offset,
                                    ap=[kap.ap[0], [0, 4]] + list(kap.ap[1:])),
                        )
                        if debug and g == 0 and s == 0:
                            nc.sync.dma_start(
                                out=d_dbg_p4[:],
                                in_=p4[:].rearrange("p a q -> p (a q)"))
                        for hh in range(2):
                            for j in range(2):
                                h = 2 * hh + j
                                nc.tensor.matmul(
                                    out=opair[hh][64 * j:64 * (j + 1), :],
                                    lhsT=v_sb[s][:, 256 * g + 64 * h:256 * g + 64 * (h + 1)],
                                    rhs=p4[:, h, :],
                                    start=(s == 0), stop=(s == NSUB - 1),
                                    tile_position=(0, 64 * j),
                                    skip_group_check=True,
                                )
                        for h in range(4):
                            nc.tensor.matmul(
                                out=r_ps[32 * h:32 * h + 1, :],
                                lhsT=ones_col[:],
                                rhs=p4[:, h, :],
                                start=(s == 0), stop=(s == NSUB - 1),
                                tile_position=(0, 32 * h),
                                skip_group_check=True,
                            )
                    # fast-evict O and r banks (release PSUM), normalize later
                    nc.vector.tensor_copy(out=oT_sb[2 * g][:], in_=o01[:])
                    nc.vector.tensor_copy(out=oT_sb[2 * g + 1][:], in_=o23[:])
                    r_sb = phD.tile([128, Q], fp32, tag="rsb")
                    nc.vector.tensor_copy(out=r_sb[:], in_=r_ps[:])
                    rinv = phD.tile([128, Q], fp32, tag="rinv")
                    nc.vector.reciprocal(out=rinv[:], in_=r_sb[:])
                    # bounce rinv rows through DRAM to partition-broadcast
                    for h in range(4):
                        nc.sync.dma_start(
                            out=d_rscratch[4 * g + h:4 * g + h + 1, :],
                            in_=rinv[32 * h:32 * h + 1, :],
                        )
                    rb = phD.tile([128, Q], fp32, tag="rb")
                    rb2 = phD.tile([128, Q], fp32, tag="rb2")
                    for hh, dst in ((0, rb), (1, rb2)):
                        for j in range(2):
                            src = d_rscratch[4 * g + 2 * hh + j:4 * g + 2 * hh + j + 1, :]
                            nc.gpsimd.dma_start(
                                out=dst[64 * j:64 * (j + 1), :],
                                in_=bass.AP(tensor=src.tensor, offset=src.offset,
                                            ap=[[0, 64]] + list(src.ap[1:])),
                            )
                    # in-place normalize (off the PSUM critical path)
                    nc.vector.tensor_mul(oT_sb[2 * g][:], oT_sb[2 * g][:], rb[:])
                    nc.vector.tensor_mul(oT_sb[2 * g + 1][:], oT_sb[2 * g + 1][:], rb2[:])
                    if debug and g == 0:
                        nc.sync.dma_start(out=d_dbg_r[:], in_=rinv[:])
                    if debug:
                        nc.sync.dma_start(
                            out=d_dbg_oT[256 * g:256 * g + 128, :],
                            in_=oT_sb[2 * g][:])
                        nc.sync.dma_start(
                            out=d_dbg_oT[256 * g + 128:256 * (g + 1), :],
                            in_=oT_sb[2 * g + 1][:])

                # ---- Phase E: out = oT.T @ WoT + cvec (overlaps D tail) ----
                out_nat = d_out[:].rearrange("(c p) d -> p c d", p=128)
                for tchunk_i in range(4):
                    ob = phE.tile([128, D], fp32, tag="ob")
                    for half in range(2):
                        ps = phE_ps.tile([128, 384], fp32, tag="ops")
                        for i in range(6):
                            nc.tensor.matmul(
                                out=ps[:],
                                lhsT=oT_sb[i][:, 128 * tchunk_i:128 * (tchunk_i + 1)],
                                rhs=woT_sb[i][:, 384 * half:384 * (half + 1)],
                                start=(i == 0), stop=(i == 5),
                            )
                        nc.vector.tensor_add(
                            out=ob[:, 384 * half:384 * (half + 1)], in0=ps[:],
                            in1=cvec_sb[:, 384 * half:384 * (half + 1)])
                    nc.sync.dma_start(out=out_nat[:, tchunk_i, :], in_=ob[:])

            scopeD.__exit__(None, None, None)
            persist_cm.__exit__(None, None, None)

    nc.compile()
    return nc, names


def kernel(**inputs):
    from concourse.bass_utils import run_bass_kernel_spmd

    nc, names, in_maps = _make_in_maps(inputs)
    res = run_bass_kernel_spmd(nc, in_maps, list(range(B)))
    out = np.stack([np.asarray(r[names["out"]], dtype=np.float32)
                    for r in res.results], axis=0)
    return out


def _make_in_maps(inputs):
    nc, names = _build()
    query = _f32(inputs["query"])
    key = _f32(inputs["key"])
    value = _f32(inputs["value"])
    mask = np.ascontiguousarray(np.asarray(inputs["attention_mask"], dtype=np.int32))
    Wq = _f32(inputs["Wq"]); bq = _f32(inputs["bq"])
    Wk = _f32(inputs["Wk"])
    Wv = _f32(inputs["Wv"]); bv = _f32(inputs["bv"])
    Wo = _f32(inputs["Wo"]); bo = _f32(inputs["bo"])
    ln_g = _f32(inputs["ln_g"]); ln_b = _f32(inputs["ln_b"])
    scale = 1.0 / np.sqrt(DH)
    wqT = _bf16((Wq * ln_g[None, :] * scale).T)
    bq_eff = (ln_b @ Wq.T + bq) * scale
    bq_arr = _f32(bq_eff.reshape(6, 128).T)
    wkT = _bf16(Wk.T)
    wvT = _bf16(Wv.T)
    woT = _bf16(Wo.T)
    cvec = _f32((bv @ Wo.T + bo).reshape(1, D))
    in_maps = []
    for b in range(B):
        in_maps.append({
            names["query"]: query[b], names["key"]: key[b],
            names["value"]: value[b], names["mask"]: mask[b],
            names["wqT"]: wqT, names["wkT"]: wkT, names["wvT"]: wvT,
            names["woT"]: woT, names["bq"]: bq_arr, names["cvec"]: cvec,
        })
    return nc, names, in_maps


def run_traced(**inputs):
    """Run with tracing enabled; returns exec_time_ns (or None)."""
    from concourse.bass_utils import run_bass_kernel_spmd
    nc, names, in_maps = _make_in_maps(inputs)
    res = run_bass_kernel_spmd(nc, in_maps, list(range(B)), trace=True)
    if res.instructions_and_trace is not None:
        print("trace:", res.instructions_and_trace[1])
    print("mean exec ns:", res.mean_exec_time_ns, "max core:", res.max_exec_time_core_id)
    if res.per_core_scope_times:
        for scope, cores in sorted(res.per_core_scope_times.items()):
            for cid, dur in cores.items():
                print(f"  scope {scope}: core{cid} {dur} ns")
    return res.exec_time_ns


if __name__ == "__main__":
    rng = np.random.default_rng(0)
    dummy = {
        "query": rng.standard_normal((B, Q, D), dtype=np.float32),
        "key": rng.standard_normal((B, KV, D), dtype=np.float32),
        "value": rng.standard_normal((B, KV, D), dtype=np.float32),
        "attention_mask": rng.integers(0, 2, (B, Q, KV)).astype(np.int32),
        "Wq": rng.standard_normal((D, D), dtype=np.float32) / 27.7,
        "bq": np.zeros(D, np.float32),
        "Wk": rng.standard_normal((D, D), dtype=np.float32) / 27.7,
        "bk": np.zeros(D, np.float32),
        "Wv": rng.standard_normal((D, D), dtype=np.float32) / 27.7,
        "bv": np.zeros(D, np.float32),
        "Wo": rng.standard_normal((D, D), dtype=np.float32) / 27.7,
        "bo": np.zeros(D, np.float32),
        "ln_g": np.ones(D, np.float32),
        "ln_b": np.zeros(D, np.float32),
    }
    out = kernel(**dummy)
    print("out", out.shape, out.dtype, float(np.abs(out).mean()))



# revision 8
# speedup vs baseline: 1.0874x; 1.0874x over previous
# Trainium2 Bass kernel: MultiHeadCrossAttentionLayer
#
# Sharding: data-parallel over batch. B=8 -> one batch element per NeuronCore,
# no collectives; gather = np.stack on host.
#
# Per-core computation (batch element b):
#   z   = standardize(query)            (LN; gamma/beta folded into Wq on host)
#   qT  = Wq_eff @ z.T + bq_eff         [768, 512]  (o on partitions)
#   kT  = Wk.T.T... = Wk @ key.T        [768, 4096] via PE-transposed key chunks
#   v   = value @ Wv.T                  [4096, 768] (kv on partitions)
#   S_T[kv, q] = kT_h.T-slices @ qT_h   per head; exp on ACT (no max needed:
#                scores ~ N(0,1), |S|max ~ 6), multiplicative mask on DVE
#   O_T[o', t] accumulated in PSUM over kv; row-sums r via ones-matmuls
#   out = (O_T / r).T @ Wo.T + cvec     (bias+norm folded; cvec = bv@Wo.T+bo)
#
# k-bias is dropped (softmax-invariant: adds a per-q constant to scores).

import functools
import numpy as np

B = 8
Q = 512
KV = 4096
D = 768
H = 12
DH = 64

NCHUNK = 16          # kv chunks for K/V projection (tokens per chunk = 256)
CTOK = KV // NCHUNK  # 256
NSUB = KV // 128     # 32 kv sub-chunks of 128 for attention
NG = 3               # head groups
GH = 4               # heads per group


def _f32(x):
    return np.ascontiguousarray(np.asarray(x, dtype=np.float32))


def _bf16(x):
    import ml_dtypes
    return np.ascontiguousarray(np.asarray(x, dtype=np.float32).astype(ml_dtypes.bfloat16))


@functools.lru_cache(maxsize=1)
def _build():
    import concourse.bass as bass
    import concourse.tile as tile
    from concourse import bacc, mybir
    from concourse.masks import make_identity

    fp32 = mybir.dt.float32
    bf16 = mybir.dt.bfloat16
    i32 = mybir.dt.int32
    AF = mybir.ActivationFunctionType
    ALU = mybir.AluOpType

    nc = bacc.Bacc(None, target_bir_lowering=False)

    names = {}

    with tile.TileContext(nc) as tc:
        with tc.tile_pool(name="dram", bufs=1, space="DRAM") as dram:
            d_query = dram.tile([Q, D], fp32, kind="ExternalInput")
            d_key = dram.tile([KV, D], fp32, kind="ExternalInput")
            d_value = dram.tile([KV, D], fp32, kind="ExternalInput")
            d_mask = dram.tile([Q, KV], i32, kind="ExternalInput")
            d_wqT = dram.tile([D, D], bf16, kind="ExternalInput")
            d_wkT = dram.tile([D, D], bf16, kind="ExternalInput")
            d_wvT = dram.tile([D, D], bf16, kind="ExternalInput")
            d_woT = dram.tile([D, D], bf16, kind="ExternalInput")
            d_bq = dram.tile([128, 6], fp32, kind="ExternalInput")
            d_cvec = dram.tile([1, D], fp32, kind="ExternalInput")
            d_out = dram.tile([Q, D], fp32, kind="ExternalOutput")
            d_rscratch = dram.tile([H, Q], fp32)

            names = dict(
                query=d_query.name, key=d_key.name, value=d_value.name,
                mask=d_mask.name, wqT=d_wqT.name, wkT=d_wkT.name,
                wvT=d_wvT.name, woT=d_woT.name, bq=d_bq.name,
                cvec=d_cvec.name, out=d_out.name,
            )

            import os
            debug = os.environ.get("BASSDBG", "0") == "1"
            if debug:
                d_dbg_qT = dram.tile([D, Q], bf16, kind="ExternalOutput")
                d_dbg_kT = dram.tile([128, KV], bf16, kind="ExternalOutput")
                d_dbg_v = dram.tile([128, D], bf16, kind="ExternalOutput")
                d_dbg_keep = dram.tile([128, Q], bf16, kind="ExternalOutput")
                d_dbg_p4 = dram.tile([128, 4 * Q], bf16, kind="ExternalOutput")
                d_dbg_oT = dram.tile([D, Q], bf16, kind="ExternalOutput")
                d_dbg_r = dram.tile([128, Q], fp32, kind="ExternalOutput")
                names.update(dbg_qT=d_dbg_qT.name, dbg_kT=d_dbg_kT.name,
                             dbg_v=d_dbg_v.name, dbg_keep=d_dbg_keep.name,
                             dbg_p4=d_dbg_p4.name, dbg_oT=d_dbg_oT.name,
                             dbg_r=d_dbg_r.name)

            # ---------------- persistent SBUF ----------------
            persist_cm = tc.tile_pool(name="persist", bufs=1)
            persist = persist_cm.__enter__()
            ident = persist.tile([128, 128], bf16)
            make_identity(nc, ident)
            ident_f = persist.tile([128, 128], fp32)
            make_identity(nc, ident_f)
            ones_col = persist.tile([128, 1], bf16)
            nc.vector.memset(ones_col, 1.0)
            bq_sb = persist.tile([128, 6], fp32)
            nc.sync.dma_start(out=bq_sb, in_=d_bq[:])
            cvec_sb = persist.tile([128, D], fp32)
            cv_ap = d_cvec[:]
            nc.gpsimd.dma_start(
                out=cvec_sb,
                in_=bass.AP(tensor=cv_ap.tensor, offset=cv_ap.offset,
                            ap=[[0, 128]] + list(cv_ap.ap[1:])),
            )

            # kT_sb: [768, 4096] bf16 as 6 tiles; v_sb: [4096, 768] as 32 tiles
            kT_sb = [persist.tile([128, KV], bf16, tag=f"kT{i}", name=f"kT{i}") for i in range(6)]
            v_sb = [persist.tile([128, D], bf16, tag=f"v{i}", name=f"v{i}") for i in range(NSUB)]
            qT_sb = [persist.tile([128, Q], bf16, tag=f"qT{i}", name=f"qT{i}") for i in range(6)]

            # =============== Phase A: LN(query) -> z.T -> qT ===============
            scopeA = nc.named_scope("phaseA_ln_q"); scopeA.__enter__()
            with (
                tc.tile_pool(name="phA", bufs=2) as phA,
                tc.tile_pool(name="phA1", bufs=1) as phA1,
                tc.tile_pool(name="phA_ps", bufs=2, space="PSUM") as phA_ps,
                tc.tile_pool(name="phA_ps2", bufs=2, space="PSUM") as phA_ps2,
            ):
                wqT_sb = [phA1.tile([128, D], bf16, tag=f"wq{i}", name=f"wq{i}") for i in range(6)]
                for i in range(6):
                    nc.sync.dma_start(out=wqT_sb[i], in_=d_wqT[128 * i:128 * (i + 1), :])
                eps_sb = phA1.tile([128, 1], fp32)
                nc.vector.memset(eps_sb, 1e-5)

                z_bf = [phA1.tile([128, D], bf16, tag=f"z{c}", name=f"z{c}") for c in range(4)]
                q_nat = d_query[:].rearrange("(c p) d -> p c d", p=128)
                for c in range(4):
                    x_t = phA.tile([128, D], fp32, tag="lnx")
                    nc.sync.dma_start(out=x_t, in_=q_nat[:, c, :])
                    # bn_stats needs free dim <= 512; use 3 subgroups of 256
                    stats = phA.tile([128, 3, 6], fp32, tag="lnst")
                    xg = x_t[:].rearrange("p (s d) -> p s d", s=3)
                    for s in range(3):
                        nc.vector.bn_stats(out=stats[:, s, :], in_=xg[:, s, :])
                    mv = phA.tile([128, 2], fp32, tag="lnmv")
                    nc.vector.bn_aggr(out=mv, in_=stats[:])
                    rstd = phA.tile([128, 1], fp32, tag="lnrs")
                    nc.scalar.activation(out=rstd, in_=mv[:, 1:2], func=AF.Sqrt,
                                         bias=eps_sb[:], scale=1.0)
                    nc.vector.reciprocal(out=rstd, in_=rstd)
                    nc.vector.tensor_scalar(
                        out=z_bf[c][:], in0=x_t[:], scalar1=mv[:, 0:1],
                        scalar2=rstd[:], op0=ALU.subtract, op1=ALU.mult,
                    )
                # transpose z -> zT (6 tiles [128, 512])
                zT = [phA1.tile([128, Q], bf16, tag=f"zT{i}", name=f"zT{i}") for i in range(6)]
                for i in range(6):
                    ps = phA_ps.tile([128, Q], bf16, tag="zTps")
                    for c in range(4):
                        nc.tensor.transpose(
                            out=ps[:, 128 * c:128 * (c + 1)],
                            in_=z_bf[c][:, 128 * i:128 * (i + 1)],
                            identity=ident[:],
                        )
                    nc.vector.tensor_copy(out=zT[i][:], in_=ps[:])
                # qT[o,:] = sum_i WqT[i, o-block].T @ zT[i]
                for o in range(6):
                    ps = phA_ps2.tile([128, Q], fp32, tag="qps")
                    for i in range(6):
                        nc.tensor.matmul(
                            out=ps[:], lhsT=wqT_sb[i][:, 128 * o:128 * (o + 1)],
                            rhs=zT[i][:], start=(i == 0), stop=(i == 5),
                        )
                    nc.vector.tensor_scalar(
                        out=qT_sb[o][:], in0=ps[:], scalar1=bq_sb[:, o:o + 1],
                        scalar2=None, op0=ALU.add,
                    )

            if debug:
                for i in range(6):
                    nc.sync.dma_start(out=d_dbg_qT[128 * i:128 * (i + 1), :],
                                      in_=qT_sb[i][:])

            scopeA.__exit__(None, None, None)
            # =============== Phase C: keepT = (1 - mask).T (early: DMA overlap) ===============
            scopeC = nc.named_scope("phaseC_mask"); scopeC.__enter__()
            keepT = [persist.tile([128, Q], bf16, tag=f"keep{s}", name=f"keep{s}") for s in range(NSUB)]
            with (
                tc.tile_pool(name="phC", bufs=3) as phC,
                tc.tile_pool(name="phC_ps", bufs=1, space="PSUM") as phC_ps,
            ):
                for s in range(NSUB):
                    mraw = phC.tile([128, 4, 128], i32, tag="mraw")
                    nc.sync.dma_start(
                        out=mraw,
                        in_=d_mask[:, 128 * s:128 * (s + 1)]
                            .rearrange("(c p) k -> p c k", p=128),
                    )
                    mb = phC.tile([128, 4, 128], bf16, tag="mbf")
                    nc.vector.tensor_scalar(
                        out=mb[:], in0=mraw[:], scalar1=-1.0, scalar2=1.0,
                        op0=ALU.mult, op1=ALU.add,
                    )
                    ps = phC_ps.tile([128, 4, 128], bf16, tag="cps")
                    for c in range(4):
                        nc.tensor.transpose(out=ps[:, c, :], in_=mb[:, c, :],
                                            identity=ident[:])
                    nc.vector.tensor_copy(
                        out=keepT[s][:].rearrange("p (c k) -> p c k", c=4), in_=ps[:])

            if debug:
                nc.sync.dma_start(out=d_dbg_keep[:], in_=keepT[0][:])

            scopeC.__exit__(None, None, None)
            # =============== Phase B: K/V projections ===============
            scopeB = nc.named_scope("phaseB_kvproj"); scopeB.__enter__()
            with (
                tc.tile_pool(name="phB1", bufs=1) as phB1,
                tc.tile_pool(name="stage", bufs=2) as stage,
                tc.tile_pool(name="tchunk", bufs=2) as tchunk,
                tc.tile_pool(name="phB_tp", bufs=3, space="PSUM") as phB_tp,
                tc.tile_pool(name="phB_k", bufs=2, space="PSUM") as phB_k,
                tc.tile_pool(name="phB_v", bufs=2, space="PSUM") as phB_v,
            ):
                wkT_sb = [phB1.tile([128, D], bf16, tag=f"wk{i}", name=f"wk{i}") for i in range(6)]
                wvT_sb = [phB1.tile([128, D], bf16, tag=f"wv{i}", name=f"wv{i}") for i in range(6)]
                for i in range(6):
                    nc.sync.dma_start(out=wkT_sb[i], in_=d_wkT[128 * i:128 * (i + 1), :])
                    nc.sync.dma_start(out=wvT_sb[i], in_=d_wvT[128 * i:128 * (i + 1), :])

                key_nat = d_key[:].rearrange("(c s p) d -> c p s d", p=128, s=CTOK // 128)
                val_nat = d_value[:].rearrange("(c s p) d -> c p s d", p=128, s=CTOK // 128)
                NS = CTOK // 128  # 2 sub-tiles of 128 tokens per chunk

                for c in range(NCHUNK):
                    kf = stage.tile([128, NS, D], fp32, tag="kf32")
                    vf = stage.tile([128, NS, D], fp32, tag="vf32")
                    nc.sync.dma_start(out=kf, in_=key_nat[c])
                    nc.sync.dma_start(out=vf, in_=val_nat[c])
                    # transpose chunks (f32 in, bf16 out via evict): [128(i), 6, CTOK]
                    keyT_c = tchunk.tile([128, 6, CTOK], bf16, tag="keyT")
                    valT_c = tchunk.tile([128, 6, CTOK], bf16, tag="valT")
                    for src, dstT in ((kf, keyT_c), (vf, valT_c)):
                        for i2 in range(0, 6, 2):  # two i-blocks -> one psum tile
                            ps = phB_tp.tile([128, 2, NS, 128], fp32, tag="tps")
                            for di in range(2):
                                for s in range(NS):
                                    nc.tensor.transpose(
                                        out=ps[:, di, s, :],
                                        in_=src[:, s, 128 * (i2 + di):128 * (i2 + di + 1)],
                                        identity=ident_f[:],
                                    )
                            # evict on ScalarE (idle during this phase); DVE is busy
                            nc.scalar.copy(
                                out=dstT[:, i2:i2 + 2, :]
                                    .rearrange("p a (s k) -> p a s k", s=NS),
                                in_=ps[:],
                            )

                    # kT += WkT.T-blocks @ keyT_c
                    for o in range(6):
                        ps = phB_k.tile([128, CTOK], fp32, tag="kps")
                        for i in range(6):
                            nc.tensor.matmul(
                                out=ps[:], lhsT=wkT_sb[i][:, 128 * o:128 * (o + 1)],
                                rhs=keyT_c[:, i, :], start=(i == 0), stop=(i == 5),
                            )
                        nc.vector.tensor_copy(
                            out=kT_sb[o][:, CTOK * c:CTOK * (c + 1)], in_=ps[:])
                    # v rows: v[t, :] = valT_c.T-blocks @ WvT
                    for s in range(NS):
                        for half in range(2):
                            ps = phB_v.tile([128, 384], fp32, tag="vps")
                            for i in range(6):
                                nc.tensor.matmul(
                                    out=ps[:],
                                    lhsT=valT_c[:, i, 128 * s:128 * (s + 1)],
                                    rhs=wvT_sb[i][:, 384 * half:384 * (half + 1)],
                                    start=(i == 0), stop=(i == 5),
                                )
                            nc.vector.tensor_copy(
                                out=v_sb[NS * c + s][:, 384 * half:384 * (half + 1)],
                                in_=ps[:])

            if debug:
                nc.sync.dma_start(out=d_dbg_kT[:], in_=kT_sb[0][:])
                nc.sync.dma_start(out=d_dbg_v[:], in_=v_sb[0][:])

            scopeB.__exit__(None, None, None)
            # =============== Phase D: attention (+ E interleaved) ===============
            scopeD = nc.named_scope("phaseD_attn"); scopeD.__enter__()
            with (
                tc.tile_pool(name="phD", bufs=3) as phD,
                tc.tile_pool(name="phD_s", bufs=2, space="PSUM") as phD_s,
                tc.tile_pool(name="phD_o", bufs=1, space="PSUM") as phD_o,
                tc.tile_pool(name="phD_r", bufs=1, space="PSUM") as phD_r,
                tc.tile_pool(name="phE", bufs=2) as phE,
                tc.tile_pool(name="phE1", bufs=1) as phE1,
                tc.tile_pool(name="phE_ps", bufs=1, space="PSUM") as phE_ps,
            ):
                woT_sb = [phE1.tile([128, D], bf16, tag=f"wo{i}", name=f"wo{i}") for i in range(6)]
                for i in range(6):
                    nc.sync.dma_start(out=woT_sb[i], in_=d_woT[128 * i:128 * (i + 1), :])
                oT_sb = [persist.tile([128, Q], bf16, tag=f"oT{i}", name=f"oT{i}") for i in range(6)]
                # 6 half-groups of 2 heads each: scores psum is [128, 2, Q] f32
                # (2 banks), double-buffered, so sub-chunk s+1's score matmuls
                # run while EXP(s) drains the other buffer.
                for hg in range(6):
                    o_ps = phD_o.tile([128, Q], fp32, tag="ops")
                    r_ps = phD_r.tile([128, Q], fp32, tag="rps")
                    kt = kT_sb[hg]  # heads 2hg (rows 0-63), 2hg+1 (rows 64-127)
                    qt = qT_sb[hg]
                    for s in range(NSUB):
                        sl = slice(128 * s, 128 * (s + 1))
                        s2 = phD_s.tile([128, 2, Q], fp32, tag="s2")
                        for j in range(2):
                            nc.tensor.matmul(
                                out=s2[:, j, :],
                                lhsT=kt[64 * j:64 * (j + 1), sl],
                                rhs=qt[64 * j:64 * (j + 1), :],
                                start=True, stop=True,
                                tile_position=(64 * j, 0),
                            )
                        e2 = phD.tile([128, 2, Q], bf16, tag="e2")
                        nc.scalar.activation(out=e2[:], in_=s2[:], func=AF.Exp)
                        p2 = phD.tile([128, 2, Q], bf16, tag="p2")
                        kap = keepT[s][:]
                        nc.vector.tensor_mul(
                            p2[:], e2[:],
                            bass.AP(tensor=kap.tensor, offset=kap.offset,
                                    ap=[kap.ap[0], [0, 2]] + list(kap.ap[1:])),
                        )
                        if debug and hg == 0 and s == 0:
                            nc.sync.dma_start(
                                out=d_dbg_p4[:, :2 * Q],
                                in_=p2[:].rearrange("p a q -> p (a q)"))
                        for j in range(2):
                            nc.tensor.matmul(
                                out=o_ps[64 * j:64 * (j + 1), :],
                                lhsT=v_sb[s][:, 128 * hg + 64 * j:128 * hg + 64 * (j + 1)],
                                rhs=p2[:, j, :],
                                start=(s == 0), stop=(s == NSUB - 1),
                                tile_position=(0, 64 * j),
                                skip_group_check=True,
                            )
                        for j in range(2):
                            nc.tensor.matmul(
                                out=r_ps[32 * j:32 * j + 1, :],
                                lhsT=ones_col[:],
                                rhs=p2[:, j, :],
                                start=(s == 0), stop=(s == NSUB - 1),
                                tile_position=(0, 32 * j),
                                skip_group_check=True,
                            )
                    # fast-evict O and r banks (release PSUM), normalize later
                    nc.vector.tensor_copy(out=oT_sb[hg][:], in_=o_ps[:])
                    r_sb = phD.tile([128, Q], fp32, tag="rsb")
                    nc.vector.tensor_copy(out=r_sb[:], in_=r_ps[:])
                    rinv = phD.tile([128, Q], fp32, tag="rinv")
                    nc.vector.reciprocal(out=rinv[:], in_=r_sb[:])
                    # bounce rinv rows through DRAM to partition-broadcast
                    for j in range(2):
                        nc.sync.dma_start(
                            out=d_rscratch[2 * hg + j:2 * hg + j + 1, :],
                            in_=rinv[32 * j:32 * j + 1, :],
                        )
                    rb = phD.tile([128, Q], fp32, tag="rb")
                    for j in range(2):
                        src = d_rscratch[2 * hg + j:2 * hg + j + 1, :]
                        nc.gpsimd.dma_start(
                            out=rb[64 * j:64 * (j + 1), :],
                            in_=bass.AP(tensor=src.tensor, offset=src.offset,
                                        ap=[[0, 64]] + list(src.ap[1:])),
                        )
                    # in-place normalize (off the PSUM critical path)
                    nc.vector.tensor_mul(oT_sb[hg][:], oT_sb[hg][:], rb[:])
                    if debug and hg == 0:
                        nc.sync.dma_start(out=d_dbg_r[:], in_=rinv[:])
                    if debug:
                        nc.sync.dma_start(
                            out=d_dbg_oT[128 * hg:128 * (hg + 1), :],
                            in_=oT_sb[hg][:])

                # ---- Phase E: out = oT.T @ WoT + cvec (overlaps D tail) ----
                out_nat = d_out[:].rearrange("(c p) d -> p c d", p=128)
                for tchunk_i in range(4):
                    ob = phE.tile([128, D], fp32, tag="ob")
                    for half in range(2):
                        ps = phE_ps.tile([128, 384], fp32, tag="ops")
                        for i in range(6):
                            nc.tensor.matmul(
                                out=ps[:],
                                lhsT=oT_sb[i][:, 128 * tchunk_i:128 * (tchunk_i + 1)],
                                rhs=woT_sb[i][:, 384 * half:384 * (half + 1)],
                                start=(i == 0), stop=(i == 5),
                            )
                        nc.vector.tensor_add(
                            out=ob[:, 384 * half:384 * (half + 1)], in0=ps[:],
                            in1=cvec_sb[:, 384 * half:384 * (half + 1)])
                    nc.sync.dma_start(out=out_nat[:, tchunk_i, :], in_=ob[:])

            scopeD.__exit__(None, None, None)
            persist_cm.__exit__(None, None, None)

    nc.compile()
    return nc, names


def kernel(**inputs):
    from concourse.bass_utils import run_bass_kernel_spmd

    nc, names, in_maps = _make_in_maps(inputs)
    res = run_bass_kernel_spmd(nc, in_maps, list(range(B)))
    out = np.stack([np.asarray(r[names["out"]], dtype=np.float32)
                    for r in res.results], axis=0)
    return out


def _make_in_maps(inputs):
    nc, names = _build()
    query = _f32(inputs["query"])
    key = _f32(inputs["key"])
    value = _f32(inputs["value"])
    mask = np.ascontiguousarray(np.asarray(inputs["attention_mask"], dtype=np.int32))
    Wq = _f32(inputs["Wq"]); bq = _f32(inputs["bq"])
    Wk = _f32(inputs["Wk"])
    Wv = _f32(inputs["Wv"]); bv = _f32(inputs["bv"])
    Wo = _f32(inputs["Wo"]); bo = _f32(inputs["bo"])
    ln_g = _f32(inputs["ln_g"]); ln_b = _f32(inputs["ln_b"])
    scale = 1.0 / np.sqrt(DH)
    wqT = _bf16((Wq * ln_g[None, :] * scale).T)
    bq_eff = (ln_b @ Wq.T + bq) * scale
    bq_arr = _f32(bq_eff.reshape(6, 128).T)
    wkT = _bf16(Wk.T)
    wvT = _bf16(Wv.T)
    woT = _bf16(Wo.T)
    cvec = _f32((bv @ Wo.T + bo).reshape(1, D))
    in_maps = []
    for b in range(B):
        in_maps.append({
            names["query"]: query[b], names["key"]: key[b],
            names["value"]: value[b], names["mask"]: mask[b],
            names["wqT"]: wqT, names["wkT"]: wkT, names["wvT"]: wvT,
            names["woT"]: woT, names["bq"]: bq_arr, names["cvec"]: cvec,
        })
    return nc, names, in_maps


def run_traced(**inputs):
    """Run with tracing enabled; returns exec_time_ns (or None)."""
    from concourse.bass_utils import run_bass_kernel_spmd
    nc, names, in_maps = _make_in_maps(inputs)
    res = run_bass_kernel_spmd(nc, in_maps, list(range(B)), trace=True)
    if res.instructions_and_trace is not None:
        print("trace:", res.instructions_and_trace[1])
    print("mean exec ns:", res.mean_exec_time_ns, "max core:", res.max_exec_time_core_id)
    if res.per_core_scope_times:
        for scope, cores in sorted(res.per_core_scope_times.items()):
            for cid, dur in cores.items():
                print(f"  scope {scope}: core{cid} {dur} ns")
    return res.exec_time_ns


if __name__ == "__main__":
    rng = np.random.default_rng(0)
    dummy = {
        "query": rng.standard_normal((B, Q, D), dtype=np.float32),
        "key": rng.standard_normal((B, KV, D), dtype=np.float32),
        "value": rng.standard_normal((B, KV, D), dtype=np.float32),
        "attention_mask": rng.integers(0, 2, (B, Q, KV)).astype(np.int32),
        "Wq": rng.standard_normal((D, D), dtype=np.float32) / 27.7,
        "bq": np.zeros(D, np.float32),
        "Wk": rng.standard_normal((D, D), dtype=np.float32) / 27.7,
        "bk": np.zeros(D, np.float32),
        "Wv": rng.standard_normal((D, D), dtype=np.float32) / 27.7,
        "bv": np.zeros(D, np.float32),
        "Wo": rng.standard_normal((D, D), dtype=np.float32) / 27.7,
        "bo": np.zeros(D, np.float32),
        "ln_g": np.ones(D, np.float32),
        "ln_b": np.zeros(D, np.float32),
    }
    out = kernel(**dummy)
    print("out", out.shape, out.dtype, float(np.abs(out).mean()))



# revision 17
# speedup vs baseline: 1.2614x; 1.1601x over previous
# Trainium2 Bass kernel: MultiHeadCrossAttentionLayer
#
# Sharding: data-parallel over batch. B=8 -> one batch element per NeuronCore,
# no collectives; gather = np.stack on host.
#
# Per-core computation (batch element b):
#   z   = standardize(query)            (LN; gamma/beta folded into Wq on host)
#   qT  = Wq_eff @ z.T + bq_eff         [768, 512]  (o on partitions)
#   kT  = Wk.T.T... = Wk @ key.T        [768, 4096] via PE-transposed key chunks
#   v   = value @ Wv.T                  [4096, 768] (kv on partitions)
#   S_T[kv, q] = kT_h.T-slices @ qT_h   per head; exp on ACT (no max needed:
#                scores ~ N(0,1), |S|max ~ 6), multiplicative mask on DVE
#   O_T[o', t] accumulated in PSUM over kv; row-sums r via ones-matmuls
#   out = (O_T / r).T @ Wo.T + cvec     (bias+norm folded; cvec = bv@Wo.T+bo)
#
# k-bias is dropped (softmax-invariant: adds a per-q constant to scores).

import functools
import numpy as np

B = 8
Q = 512
KV = 4096
D = 768
H = 12
DH = 64

NCHUNK = 16          # kv chunks for K/V projection (tokens per chunk = 256)
CTOK = KV // NCHUNK  # 256
NSUB = KV // 128     # 32 kv sub-chunks of 128 for attention
NG = 3               # head groups
GH = 4               # heads per group


def _f32(x):
    return np.ascontiguousarray(np.asarray(x, dtype=np.float32))


def _bf16(x):
    import ml_dtypes
    return np.ascontiguousarray(np.asarray(x, dtype=np.float32).astype(ml_dtypes.bfloat16))


@functools.lru_cache(maxsize=1)
def _build():
    import concourse.bass as bass
    import concourse.tile as tile
    from concourse import bacc, mybir
    from concourse.masks import make_identity

    fp32 = mybir.dt.float32
    bf16 = mybir.dt.bfloat16
    i32 = mybir.dt.int32
    AF = mybir.ActivationFunctionType
    ALU = mybir.AluOpType

    nc = bacc.Bacc(None, target_bir_lowering=False)

    names = {}

    with tile.TileContext(nc) as tc:
        with tc.tile_pool(name="dram", bufs=1, space="DRAM") as dram:
            d_query = dram.tile([Q, D], fp32, kind="ExternalInput")
            d_key = dram.tile([KV, D], fp32, kind="ExternalInput")
            d_value = dram.tile([KV, D], fp32, kind="ExternalInput")
            d_mask = dram.tile([Q, KV], i32, kind="ExternalInput")
            d_wqT = dram.tile([D, D], bf16, kind="ExternalInput")
            d_wkT = dram.tile([D, D], bf16, kind="ExternalInput")
            d_wvT = dram.tile([D, D], bf16, kind="ExternalInput")
            d_woT = dram.tile([D, D], bf16, kind="ExternalInput")
            d_bq = dram.tile([128, 6], fp32, kind="ExternalInput")
            d_cvec = dram.tile([1, D], fp32, kind="ExternalInput")
            d_out = dram.tile([Q, D], fp32, kind="ExternalOutput")
            d_rscratch = dram.tile([H, Q], fp32)

            names = dict(
                query=d_query.name, key=d_key.name, value=d_value.name,
                mask=d_mask.name, wqT=d_wqT.name, wkT=d_wkT.name,
                wvT=d_wvT.name, woT=d_woT.name, bq=d_bq.name,
                cvec=d_cvec.name, out=d_out.name,
            )

            import os
            debug = os.environ.get("BASSDBG", "0") == "1"
            if debug:
                d_dbg_qT = dram.tile([D, Q], bf16, kind="ExternalOutput")
                d_dbg_kT = dram.tile([128, KV], bf16, kind="ExternalOutput")
                d_dbg_v = dram.tile([128, D], bf16, kind="ExternalOutput")
                d_dbg_keep = dram.tile([128, Q], bf16, kind="ExternalOutput")
                d_dbg_p4 = dram.tile([128, 4 * Q], bf16, kind="ExternalOutput")
                d_dbg_oT = dram.tile([D, Q], bf16, kind="ExternalOutput")
                d_dbg_r = dram.tile([128, Q], fp32, kind="ExternalOutput")
                names.update(dbg_qT=d_dbg_qT.name, dbg_kT=d_dbg_kT.name,
                             dbg_v=d_dbg_v.name, dbg_keep=d_dbg_keep.name,
                             dbg_p4=d_dbg_p4.name, dbg_oT=d_dbg_oT.name,
                             dbg_r=d_dbg_r.name)

            # ---------------- persistent SBUF ----------------
            persist_cm = tc.tile_pool(name="persist", bufs=1)
            persist = persist_cm.__enter__()
            ident = persist.tile([128, 128], bf16)
            make_identity(nc, ident)
            ident_f = persist.tile([128, 128], fp32)
            make_identity(nc, ident_f)
            ones_col = persist.tile([128, 1], bf16)
            nc.vector.memset(ones_col, 1.0)
            bq_sb = persist.tile([128, 6], fp32)
            nc.sync.dma_start(out=bq_sb, in_=d_bq[:])
            cvec_sb = persist.tile([128, D], fp32)
            cv_ap = d_cvec[:]
            nc.gpsimd.dma_start(
                out=cvec_sb,
                in_=bass.AP(tensor=cv_ap.tensor, offset=cv_ap.offset,
                            ap=[[0, 128]] + list(cv_ap.ap[1:])),
            )

            # kT_sb: [768, 4096] bf16 as 6 tiles.
            # v_sb: [4096, 780] as 32 tiles: per head h, cols [65h, 65h+64) hold
            # v-head h and col 65h+64 holds 1.0 -- the AV matmul with this
            # augmented [128, 65] stationary emits [O_h; rowsum_h] in one shot.
            kT_sb = [persist.tile([128, KV], bf16, tag=f"kT{i}", name=f"kT{i}") for i in range(6)]
            v_sb = [persist.tile([128, 65 * H], bf16, tag=f"v{i}", name=f"v{i}") for i in range(NSUB)]
            for i in range(NSUB):
                nc.vector.memset(
                    v_sb[i][:].rearrange("p (h e) -> p h e", e=65)[:, :, 64:65], 1.0)
            qT_sb = [persist.tile([128, Q], bf16, tag=f"qT{i}", name=f"qT{i}") for i in range(6)]

            # =============== Phase A: LN(query) -> z.T -> qT ===============
            scopeA = nc.named_scope("phaseA_ln_q"); scopeA.__enter__()
            with (
                tc.tile_pool(name="phA", bufs=2) as phA,
                tc.tile_pool(name="phA1", bufs=1) as phA1,
                tc.tile_pool(name="phA_ps", bufs=2, space="PSUM") as phA_ps,
                tc.tile_pool(name="phA_ps2", bufs=2, space="PSUM") as phA_ps2,
            ):
                wqT_sb = [phA1.tile([128, D], bf16, tag=f"wq{i}", name=f"wq{i}") for i in range(6)]
                for i in range(6):
                    nc.sync.dma_start(out=wqT_sb[i], in_=d_wqT[128 * i:128 * (i + 1), :])
                eps_sb = phA1.tile([128, 1], fp32)
                nc.vector.memset(eps_sb, 1e-5)

                z_bf = [phA1.tile([128, D], bf16, tag=f"z{c}", name=f"z{c}") for c in range(4)]
                q_nat = d_query[:].rearrange("(c p) d -> p c d", p=128)
                for c in range(4):
                    x_t = phA.tile([128, D], fp32, tag="lnx")
                    nc.sync.dma_start(out=x_t, in_=q_nat[:, c, :])
                    # bn_stats needs free dim <= 512; use 3 subgroups of 256
                    stats = phA.tile([128, 3, 6], fp32, tag="lnst")
                    xg = x_t[:].rearrange("p (s d) -> p s d", s=3)
                    for s in range(3):
                        nc.vector.bn_stats(out=stats[:, s, :], in_=xg[:, s, :])
                    mv = phA.tile([128, 2], fp32, tag="lnmv")
                    nc.vector.bn_aggr(out=mv, in_=stats[:])
                    rstd = phA.tile([128, 1], fp32, tag="lnrs")
                    nc.scalar.activation(out=rstd, in_=mv[:, 1:2], func=AF.Sqrt,
                                         bias=eps_sb[:], scale=1.0)
                    nc.vector.reciprocal(out=rstd, in_=rstd)
                    nc.vector.tensor_scalar(
                        out=z_bf[c][:], in0=x_t[:], scalar1=mv[:, 0:1],
                        scalar2=rstd[:], op0=ALU.subtract, op1=ALU.mult,
                    )
                # transpose z -> zT (6 tiles [128, 512])
                zT = [phA1.tile([128, Q], bf16, tag=f"zT{i}", name=f"zT{i}") for i in range(6)]
                for i in range(6):
                    ps = phA_ps.tile([128, Q], bf16, tag="zTps")
                    for c in range(4):
                        nc.tensor.transpose(
                            out=ps[:, 128 * c:128 * (c + 1)],
                            in_=z_bf[c][:, 128 * i:128 * (i + 1)],
                            identity=ident[:],
                        )
                    nc.vector.tensor_copy(out=zT[i][:], in_=ps[:])
                # qT[o,:] = sum_i WqT[i, o-block].T @ zT[i]
                for o in range(6):
                    ps = phA_ps2.tile([128, Q], fp32, tag="qps")
                    for i in range(6):
                        nc.tensor.matmul(
                            out=ps[:], lhsT=wqT_sb[i][:, 128 * o:128 * (o + 1)],
                            rhs=zT[i][:], start=(i == 0), stop=(i == 5),
                        )
                    nc.vector.tensor_scalar(
                        out=qT_sb[o][:], in0=ps[:], scalar1=bq_sb[:, o:o + 1],
                        scalar2=None, op0=ALU.add,
                    )

            if debug:
                for i in range(6):
                    nc.sync.dma_start(out=d_dbg_qT[128 * i:128 * (i + 1), :],
                                      in_=qT_sb[i][:])

            scopeA.__exit__(None, None, None)
            # ========= Phase B: K/V projections + mask (C) interleaved =========
            # Mask DMAs are paced per-chunk so the 8MB int32 mask streams under
            # the PE-bound projection work instead of hogging the DMA queues.
            keepT = [persist.tile([128, Q], bf16, tag=f"keep{s}", name=f"keep{s}") for s in range(NSUB)]
            scopeB = nc.named_scope("phaseB_kvproj"); scopeB.__enter__()
            with (
                tc.tile_pool(name="phB1", bufs=1) as phB1,
                tc.tile_pool(name="stage", bufs=2) as stage,
                tc.tile_pool(name="tchunk", bufs=2) as tchunk,
                tc.tile_pool(name="phC", bufs=3) as phC,
                tc.tile_pool(name="phB_tp", bufs=3, space="PSUM") as phB_tp,
                tc.tile_pool(name="phB_k", bufs=2, space="PSUM") as phB_k,
                tc.tile_pool(name="phB_v", bufs=2, space="PSUM") as phB_v,
                tc.tile_pool(name="phC_ps", bufs=1, space="PSUM") as phC_ps,
            ):
                wkT_sb = [phB1.tile([128, D], bf16, tag=f"wk{i}", name=f"wk{i}") for i in range(6)]
                wvT_sb = [phB1.tile([128, D], bf16, tag=f"wv{i}", name=f"wv{i}") for i in range(6)]
                for i in range(6):
                    nc.sync.dma_start(out=wkT_sb[i], in_=d_wkT[128 * i:128 * (i + 1), :])
                    nc.sync.dma_start(out=wvT_sb[i], in_=d_wvT[128 * i:128 * (i + 1), :])

                key_nat = d_key[:].rearrange("(c s p) d -> c p s d", p=128, s=CTOK // 128)
                val_nat = d_value[:].rearrange("(c s p) d -> c p s d", p=128, s=CTOK // 128)
                NS = CTOK // 128  # 2 sub-tiles of 128 tokens per chunk

                for c in range(NCHUNK):
                    kf = stage.tile([128, NS, D], fp32, tag="kf32")
                    vf = stage.tile([128, NS, D], fp32, tag="vf32")
                    nc.sync.dma_start(out=kf, in_=key_nat[c])
                    nc.sync.dma_start(out=vf, in_=val_nat[c])
                    # mask sub-chunks 2c, 2c+1 (phase C work, paced per chunk)
                    mraws = []
                    for s in (2 * c, 2 * c + 1):
                        mraw = phC.tile([128, 4, 128], i32, tag="mraw")
                        nc.sync.dma_start(
                            out=mraw,
                            in_=d_mask[:, 128 * s:128 * (s + 1)]
                                .rearrange("(c p) k -> p c k", p=128),
                        )
                        mraws.append(mraw)
                    # transpose chunks (f32 in, bf16 out via evict): [128(i), 6, CTOK]
                    keyT_c = tchunk.tile([128, 6, CTOK], bf16, tag="keyT")
                    valT_c = tchunk.tile([128, 6, CTOK], bf16, tag="valT")
                    for src, dstT in ((kf, keyT_c), (vf, valT_c)):
                        for i2 in range(0, 6, 2):  # two i-blocks -> one psum tile
                            ps = phB_tp.tile([128, 2, NS, 128], fp32, tag="tps")
                            for di in range(2):
                                for s in range(NS):
                                    nc.tensor.transpose(
                                        out=ps[:, di, s, :],
                                        in_=src[:, s, 128 * (i2 + di):128 * (i2 + di + 1)],
                                        identity=ident_f[:],
                                    )
                            # evict on ScalarE (idle during this phase); DVE is busy
                            nc.scalar.copy(
                                out=dstT[:, i2:i2 + 2, :]
                                    .rearrange("p a (s k) -> p a s k", s=NS),
                                in_=ps[:],
                            )

                    # kT += WkT.T-blocks @ keyT_c
                    for o in range(6):
                        ps = phB_k.tile([128, CTOK], fp32, tag="kps")
                        for i in range(6):
                            nc.tensor.matmul(
                                out=ps[:], lhsT=wkT_sb[i][:, 128 * o:128 * (o + 1)],
                                rhs=keyT_c[:, i, :], start=(i == 0), stop=(i == 5),
                            )
                        nc.vector.tensor_copy(
                            out=kT_sb[o][:, CTOK * c:CTOK * (c + 1)], in_=ps[:])
                    # v rows: v[t, :] = valT_c.T-blocks @ WvT (strided evict into
                    # the 65-col-per-head augmented layout)
                    for s in range(NS):
                        for half in range(2):
                            ps = phB_v.tile([128, 384], fp32, tag="vps")
                            for i in range(6):
                                nc.tensor.matmul(
                                    out=ps[:],
                                    lhsT=valT_c[:, i, 128 * s:128 * (s + 1)],
                                    rhs=wvT_sb[i][:, 384 * half:384 * (half + 1)],
                                    start=(i == 0), stop=(i == 5),
                                )
                            vdst = (v_sb[NS * c + s][:, 390 * half:390 * (half + 1)]
                                    .rearrange("p (h e) -> p h e", e=65)[:, :, 0:64])
                            nc.vector.tensor_copy(
                                out=vdst,
                                in_=ps[:].rearrange("p (h d) -> p h d", d=64))
                    # mask: convert to keep=1-mask (bf16) and transpose
                    for si, s in enumerate((2 * c, 2 * c + 1)):
                        mb = phC.tile([128, 4, 128], bf16, tag="mbf")
                        nc.vector.tensor_scalar(
                            out=mb[:], in0=mraws[si][:], scalar1=-1.0, scalar2=1.0,
                            op0=ALU.mult, op1=ALU.add,
                        )
                        psm = phC_ps.tile([128, 4, 128], bf16, tag="cps")
                        for cc in range(4):
                            nc.tensor.transpose(out=psm[:, cc, :], in_=mb[:, cc, :],
                                                identity=ident[:])
                        nc.vector.tensor_copy(
                            out=keepT[s][:].rearrange("p (c k) -> p c k", c=4),
                            in_=psm[:])

            if debug:
                nc.sync.dma_start(out=d_dbg_kT[:], in_=kT_sb[0][:])
                nc.sync.dma_start(out=d_dbg_v[:], in_=v_sb[0][:])

            scopeB.__exit__(None, None, None)
            # =============== Phase D: attention (+ E interleaved) ===============
            scopeD = nc.named_scope("phaseD_attn"); scopeD.__enter__()
            with (
                tc.tile_pool(name="phD", bufs=3) as phD,
                tc.tile_pool(name="phD_s", bufs=2, space="PSUM") as phD_s,
                tc.tile_pool(name="phD_o", bufs=1, space="PSUM") as phD_o,
                tc.tile_pool(name="phE", bufs=2) as phE,
                tc.tile_pool(name="phE1", bufs=1) as phE1,
                tc.tile_pool(name="phE_ps", bufs=1, space="PSUM") as phE_ps,
            ):
                woT_sb = [phE1.tile([128, D], bf16, tag=f"wo{i}", name=f"wo{i}") for i in range(6)]
                for i in range(6):
                    nc.sync.dma_start(out=woT_sb[i], in_=d_woT[128 * i:128 * (i + 1), :])
                oT_sb = [persist.tile([128, Q], bf16, tag=f"oT{i}", name=f"oT{i}") for i in range(6)]
                # 6 half-groups of 2 heads each: scores psum is [128, 2, Q] f32
                # (2 banks), double-buffered, so sub-chunk s+1's score matmuls
                # run while EXP(s) drains the other buffer.
                for hg in range(6):
                    # augmented AV: out rows 0-63 = O_head, row 64 = rowsum
                    o_psA = phD_o.tile([65, Q], fp32, tag="opsA")
                    o_psB = phD_o.tile([65, Q], fp32, tag="opsB")
                    kt = kT_sb[hg]  # heads 2hg (rows 0-63), 2hg+1 (rows 64-127)
                    qt = qT_sb[hg]
                    for s in range(NSUB):
                        sl = slice(128 * s, 128 * (s + 1))
                        s2 = phD_s.tile([128, 2, Q], fp32, tag="s2")
                        for j in range(2):
                            nc.tensor.matmul(
                                out=s2[:, j, :],
                                lhsT=kt[64 * j:64 * (j + 1), sl],
                                rhs=qt[64 * j:64 * (j + 1), :],
                                start=True, stop=True,
                                tile_position=(64 * j, 0),
                            )
                        e2 = phD.tile([128, 2, Q], bf16, tag="e2")
                        nc.scalar.activation(out=e2[:], in_=s2[:], func=AF.Exp)
                        p2 = phD.tile([128, 2, Q], bf16, tag="p2")
                        kap = keepT[s][:]
                        nc.vector.tensor_mul(
                            p2[:], e2[:],
                            bass.AP(tensor=kap.tensor, offset=kap.offset,
                                    ap=[kap.ap[0], [0, 2]] + list(kap.ap[1:])),
                        )
                        if debug and hg == 0 and s == 0:
                            nc.sync.dma_start(
                                out=d_dbg_p4[:, :2 * Q],
                                in_=p2[:].rearrange("p a q -> p (a q)"))
                        for j, ops in ((0, o_psA), (1, o_psB)):
                            h = 2 * hg + j
                            nc.tensor.matmul(
                                out=ops[:],
                                lhsT=v_sb[s][:, 65 * h:65 * (h + 1)],
                                rhs=p2[:, j, :],
                                start=(s == 0), stop=(s == NSUB - 1),
                                skip_group_check=True,
                            )
                    # evict O halves and rowsum rows (release PSUM banks)
                    nc.vector.tensor_copy(out=oT_sb[hg][0:64, :], in_=o_psA[0:64, :])
                    nc.vector.tensor_copy(out=oT_sb[hg][64:128, :], in_=o_psB[0:64, :])
                    # 1/r via ACT: lg=ln(r); broadcast; exp(-lg). (DVE reciprocal
                    # is an 8-cycle/elem iterative divide -- far slower.) ACT
                    # reads the rowsum rows straight out of PSUM.
                    lg2 = phD.tile([128, Q], fp32, tag="lg2")
                    nc.scalar.activation(out=lg2[0:1, :], in_=o_psA[64:65, :],
                                         func=AF.Ln)
                    nc.scalar.activation(out=lg2[32:33, :], in_=o_psB[64:65, :],
                                         func=AF.Ln)
                    for j, row in ((0, 0), (1, 32)):
                        nc.sync.dma_start(
                            out=d_rscratch[2 * hg + j:2 * hg + j + 1, :],
                            in_=lg2[row:row + 1, :],
                        )
                    rbl = phD.tile([128, Q], fp32, tag="rbl")
                    for j in range(2):
                        src = d_rscratch[2 * hg + j:2 * hg + j + 1, :]
                        nc.gpsimd.dma_start(
                            out=rbl[64 * j:64 * (j + 1), :],
                            in_=bass.AP(tensor=src.tensor, offset=src.offset,
                                        ap=[[0, 64]] + list(src.ap[1:])),
                        )
                    rb = phD.tile([128, Q], fp32, tag="rb")
                    nc.scalar.activation(out=rb[:], in_=rbl[:], func=AF.Exp,
                                         scale=-1.0)
                    # in-place normalize (off the PSUM critical path)
                    nc.vector.tensor_mul(oT_sb[hg][:], oT_sb[hg][:], rb[:])
                    if debug:
                        nc.sync.dma_start(
                            out=d_dbg_oT[128 * hg:128 * (hg + 1), :],
                            in_=oT_sb[hg][:])

                # ---- Phase E: out = oT.T @ WoT + cvec (overlaps D tail) ----
                out_nat = d_out[:].rearrange("(c p) d -> p c d", p=128)
                for tchunk_i in range(4):
                    ob = phE.tile([128, D], fp32, tag="ob")
                    for half in range(2):
                        ps = phE_ps.tile([128, 384], fp32, tag="ops")
                        for i in range(6):
                            nc.tensor.matmul(
                                out=ps[:],
                                lhsT=oT_sb[i][:, 128 * tchunk_i:128 * (tchunk_i + 1)],
                                rhs=woT_sb[i][:, 384 * half:384 * (half + 1)],
                                start=(i == 0), stop=(i == 5),
                            )
                        nc.vector.tensor_add(
                            out=ob[:, 384 * half:384 * (half + 1)], in0=ps[:],
                            in1=cvec_sb[:, 384 * half:384 * (half + 1)])
                    nc.sync.dma_start(out=out_nat[:, tchunk_i, :], in_=ob[:])

            scopeD.__exit__(None, None, None)
            persist_cm.__exit__(None, None, None)

    nc.compile()
    return nc, names


def kernel(**inputs):
    from concourse.bass_utils import run_bass_kernel_spmd

    nc, names, in_maps = _make_in_maps(inputs)
    res = run_bass_kernel_spmd(nc, in_maps, list(range(B)))
    out = np.stack([np.asarray(r[names["out"]], dtype=np.float32)
                    for r in res.results], axis=0)
    return out


def _make_in_maps(inputs):
    nc, names = _build()
    query = _f32(inputs["query"])
    key = _f32(inputs["key"])
    value = _f32(inputs["value"])
    mask = np.ascontiguousarray(np.asarray(inputs["attention_mask"], dtype=np.int32))
    Wq = _f32(inputs["Wq"]); bq = _f32(inputs["bq"])
    Wk = _f32(inputs["Wk"])
    Wv = _f32(inputs["Wv"]); bv = _f32(inputs["bv"])
    Wo = _f32(inputs["Wo"]); bo = _f32(inputs["bo"])
    ln_g = _f32(inputs["ln_g"]); ln_b = _f32(inputs["ln_b"])
    scale = 1.0 / np.sqrt(DH)
    wqT = _bf16((Wq * ln_g[None, :] * scale).T)
    bq_eff = (ln_b @ Wq.T + bq) * scale
    bq_arr = _f32(bq_eff.reshape(6, 128).T)
    wkT = _bf16(Wk.T)
    wvT = _bf16(Wv.T)
    woT = _bf16(Wo.T)
    cvec = _f32((bv @ Wo.T + bo).reshape(1, D))
    in_maps = []
    for b in range(B):
        in_maps.append({
            names["query"]: query[b], names["key"]: key[b],
            names["value"]: value[b], names["mask"]: mask[b],
            names["wqT"]: wqT, names["wkT"]: wkT, names["wvT"]: wvT,
            names["woT"]: woT, names["bq"]: bq_arr, names["cvec"]: cvec,
        })
    return nc, names, in_maps


def run_traced(**inputs):
    """Run with tracing enabled; returns exec_time_ns (or None)."""
    from concourse.bass_utils import run_bass_kernel_spmd
    nc, names, in_maps = _make_in_maps(inputs)
    res = run_bass_kernel_spmd(nc, in_maps, list(range(B)), trace=True)
    if res.instructions_and_trace is not None:
        print("trace:", res.instructions_and_trace[1])
    print("mean exec ns:", res.mean_exec_time_ns, "max core:", res.max_exec_time_core_id)
    if res.per_core_scope_times:
        for scope, cores in sorted(res.per_core_scope_times.items()):
            for cid, dur in cores.items():
                print(f"  scope {scope}: core{cid} {dur} ns")
    return res.exec_time_ns


if __name__ == "__main__":
    rng = np.random.default_rng(0)
    dummy = {
        "query": rng.standard_normal((B, Q, D), dtype=np.float32),
        "key": rng.standard_normal((B, KV, D), dtype=np.float32),
        "value": rng.standard_normal((B, KV, D), dtype=np.float32),
        "attention_mask": rng.integers(0, 2, (B, Q, KV)).astype(np.int32),
        "Wq": rng.standard_normal((D, D), dtype=np.float32) / 27.7,
        "bq": np.zeros(D, np.float32),
        "Wk": rng.standard_normal((D, D), dtype=np.float32) / 27.7,
        "bk": np.zeros(D, np.float32),
        "Wv": rng.standard_normal((D, D), dtype=np.float32) / 27.7,
        "bv": np.zeros(D, np.float32),
        "Wo": rng.standard_normal((D, D), dtype=np.float32) / 27.7,
        "bo": np.zeros(D, np.float32),
        "ln_g": np.ones(D, np.float32),
        "ln_b": np.zeros(D, np.float32),
    }
    out = kernel(**dummy)
    print("out", out.shape, out.dtype, float(np.abs(out).mean()))

